# revision 8
# baseline (speedup 1.0000x reference)
"""Multi-headed attention (pre-LN, quirk-wired) Trainium2 Bass kernel.

Sharding: 8 cores = 2 batches x 4 head-groups (4 heads each).
Per core: full-batch LN-folded projections (its head slice), scores^T
layout attention with ones-column softmax denominators, Wo partial,
ReduceScatter over the 4-core batch group, residual added post-RS.

reference semantics:
  kn,qn,vn = LN(k),LN(q),LN(v)   (ddof=1 std, eps added to std, affine a2,b2)
  query = kn@Wq+bq ; key = qn@Wk+bk ; value = vn@Wv+bv   (note stream quirk)
  out = softmax(mask(QK^T/8)) @ V  -> @Wo + bo + vn
"""
import math
import numpy as np
import ml_dtypes

import concourse.bass as bass
import concourse.tile as tile
from concourse import bacc, mybir
from concourse.bass_utils import run_bass_kernel_spmd
from concourse.masks import make_identity

BF = ml_dtypes.bfloat16
B, S, D, H = 2, 2048, 1024, 16
DK = D // H            # 64
NCORES = 8
HG = 4                 # head-groups per batch
HPG = H // HG          # 4 heads per core
DHG = HPG * DK         # 256 head-dim slice per core
EPS = 1e-6
P = 128
NTT = S // P           # 16 token tiles
NQS = 4                # query slices of 512
QS = S // NQS          # 512
VAR_C = D / (D - 1.0)  # ddof=1 correction

_CACHE = {}


def _build(has_a2, has_bias_out):
    nc = bacc.Bacc("TRN2", target_bir_lowering=False, debug=False,
                   num_devices=NCORES)
    f32, bf16 = mybir.dt.float32, mybir.dt.bfloat16

    dram_in = {}
    for nm, shape, dt in [
        ("xk", [S, D], f32), ("xq", [S, D], f32), ("xv", [S, D], f32),
        ("v_own", [NQS, P, D], f32),
        ("mask_bf", [S, S], bf16),
        ("wq", [D, DHG], bf16), ("wk", [D, DHG], bf16), ("wv", [D, DHG], bf16),
        ("wo", [DHG, D], bf16),
        ("cq", [2, DHG], bf16), ("ck", [2, DHG], bf16), ("cv", [2, DHG], bf16),
        ("a2f", [1, D], f32), ("bias_out", [1, D], f32),
    ]:
        dram_in[nm] = nc.dram_tensor(nm, shape, dt, kind="ExternalInput").ap()
    out_shard = nc.dram_tensor("out_shard", [NQS, P, D], f32,
                               kind="ExternalOutput").ap()

    with tile.TileContext(nc, trace_sim=False) as tc:
        with tc.tile_pool(name="const", bufs=1) as constp, \
             tc.tile_pool(name="persist", bufs=1) as persist, \
             tc.tile_pool(name="dram", bufs=1, space="DRAM") as dramp:

            x_scratch = dramp.tile([S, D], bf16, tag="xscr")
            ident = constp.tile([P, P], f32)
            make_identity(nc, ident)
            identb = constp.tile([P, P], bf16)
            nc.vector.tensor_copy(identb[:], ident[:])

            # weights to SBUF  [128, kt, DHG] etc.
            w_sb = {}
            for nm in ["wq", "wk", "wv"]:
                t = persist.tile([P, D // P, DHG], bf16, tag=f"w_{nm}", name=f"w_{nm}")
                nc.sync.dma_start(t[:], dram_in[nm].rearrange(
                    "(kt p) n -> p kt n", p=P))
                w_sb[nm] = t
            wo_sb = persist.tile([P, DHG // P, D], bf16, tag="w_wo")
            nc.sync.dma_start(wo_sb[:], dram_in["wo"].rearrange(
                "(kt p) n -> p kt n", p=P))
            c_sb = {}
            for nm in ["cq", "ck", "cv"]:
                t = persist.tile([2, DHG], bf16, tag=f"c_{nm}", name=f"c_{nm}")
                nc.sync.dma_start(t[:], dram_in[nm][:])
                c_sb[nm] = t

            # persistent activation tensors
            qT = persist.tile([P, DHG // P, S], bf16, tag="qT")   # Q^T [dk, tok]
            kT = persist.tile([P, DHG // P, S], bf16, tag="kT")   # K^T [dk, tok]
            vhat = persist.tile([P, NTT, HPG, DK + 1], bf16, tag="vhat")
            nc.vector.memset(vhat[:], 0.0)
            nc.vector.memset(vhat[:, :, :, DK:DK + 1], 1.0)
            rows = persist.tile([2, S], bf16, tag="rows")         # [-mean; ones]
            nc.vector.memset(rows[:], 1.0)
            rinv_bc = {}
            for nm in ["k", "q"]:
                rinv_bc[nm] = persist.tile([P, S], f32, tag=f"rinvbc_{nm}", name=f"rinvbc_{nm}")

            # ---------------- Phase A: stats + transpose + projections -------
            for idx, (xnm, wnm, cnm) in enumerate([
                    ("xk", "wq", "cq"), ("xq", "wk", "ck"), ("xv", "wv", "cv")]):
                with tc.tile_pool(name=f"pa_{idx}", bufs=3) as pa, \
                     tc.tile_pool(name=f"paps_{idx}", bufs=2, space="PSUM") as paps, \
                     tc.tile_pool(name=f"pap2_{idx}", bufs=3, space="PSUM") as pap2:
                    rinv_row = pa.tile([1, S], f32, tag="rinv_row", bufs=1)
                    rinv_cols = pa.tile([P, NTT], f32, tag="rinv_cols", bufs=1)
                    for tt in range(NTT):
                        xt = pa.tile([P, D], f32, tag="xt")
                        nc.sync.dma_start(xt[:], dram_in[xnm][tt * P:(tt + 1) * P, :])
                        # cast + stash to scratch for DMA-transpose
                        xb = pa.tile([P, D], bf16, tag="xb")
                        nc.gpsimd.tensor_copy(xb[:], xt[:])
                        nc.sync.dma_start(x_scratch[tt * P:(tt + 1) * P, :], xb[:])
                        # LN stats
                        st = pa.tile([P, 2, 6], f32, tag="bnst")
                        xr = xt[:].rearrange("p (n f) -> p n f", f=512)
                        nc.vector.bn_stats(out=st[:, 0], in_=xr[:, 0])
                        nc.vector.bn_stats(out=st[:, 1], in_=xr[:, 1])
                        mv = pa.tile([P, 2], f32, tag="mv")
                        nc.vector.bn_aggr(out=mv[:], in_=st[:])
                        pack = pa.tile([P, 2], f32, tag="pack")
                        # pack[:,0] = -mean ; pack[:,1] = 1/(sqrt(var*c)+eps)
                        nc.vector.tensor_scalar(out=pack[:, 0:1], in0=mv[:, 0:1],
                                                scalar1=-1.0, scalar2=None,
                                                op0=mybir.AluOpType.mult)
                        sd = pa.tile([P, 1], f32, tag="sd")
                        nc.scalar.activation(sd[:], mv[:, 1:2],
                                             mybir.ActivationFunctionType.Sqrt,
                                             scale=VAR_C)
                        nc.vector.tensor_scalar(out=sd[:], in0=sd[:],
                                                scalar1=EPS, scalar2=None,
                                                op0=mybir.AluOpType.add)
                        nc.vector.reciprocal(pack[:, 1:2], sd[:])
                        nc.gpsimd.tensor_copy(rinv_cols[:, tt:tt + 1], pack[:, 1:2])
                        # transpose stats to rows (two base-0 transposes)
                        pst0 = pap2.tile([1, P], f32, tag="pst0")
                        nc.tensor.transpose(pst0[:], pack[:, 0:1], ident[:])
                        nc.scalar.copy(rows[0:1, tt * P:(tt + 1) * P], pst0[:])
                        pst1 = pap2.tile([1, P], f32, tag="pst1")
                        nc.tensor.transpose(pst1[:], pack[:, 1:2], ident[:])
                        nc.scalar.copy(rinv_row[:, tt * P:(tt + 1) * P], pst1[:])
                    # x^T via DMA-transpose from scratch  [128, kt, S]
                    xT = pa.tile([P, D // P, S], bf16, tag="xT", bufs=1)
                    for kt in range(D // P):
                        nc.sync.dma_start(
                            xT[:, kt], x_scratch[:, kt * P:(kt + 1) * P],
                            transpose=True)
                    if idx < 2:
                        # rinv broadcast tile for Q/K evac
                        nc.gpsimd.partition_broadcast(
                            rinv_bc["k" if idx == 0 else "q"][:], rinv_row[:])
                        # projection -> feature-major [dk, tok]
                        dstT = qT if idx == 0 else kT
                        for m in range(DHG // P):
                            for nn in range(NQS):
                                ps = paps.tile([P, QS], f32, tag="projps")
                                for kt in range(D // P):
                                    nc.tensor.matmul(
                                        ps[:],
                                        w_sb[wnm][:, kt, m * P:(m + 1) * P],
                                        xT[:, kt, nn * QS:(nn + 1) * QS],
                                        start=(kt == 0), stop=False)
                                nc.tensor.matmul(
                                    ps[:], c_sb[cnm][:, m * P:(m + 1) * P],
                                    rows[:, nn * QS:(nn + 1) * QS],
                                    start=False, stop=True)
                                nc.vector.tensor_mul(
                                    dstT[:, m, nn * QS:(nn + 1) * QS], ps[:],
                                    rinv_bc["k" if idx == 0 else "q"][:, nn * QS:(nn + 1) * QS])
                    else:
                        # V projection -> token-major [tok, dk], scaled by rinv col
                        for m in range(NTT):
                            ps = paps.tile([P, DHG], f32, tag="projps")
                            for kt in range(D // P):
                                nc.tensor.matmul(
                                    ps[:], xT[:, kt, m * P:(m + 1) * P],
                                    w_sb[wnm][:, kt, :],
                                    start=(kt == 0), stop=False)
                            nc.tensor.matmul(
                                ps[:], rows[:, m * P:(m + 1) * P], c_sb[cnm][:],
                                start=False, stop=True)
                            nc.vector.tensor_scalar(
                                out=vhat[:, m, :, 0:DK],
                                in0=ps[:].rearrange("p (h d) -> p h d", h=HPG),
                                scalar1=rinv_cols[:, m:m + 1], scalar2=None,
                                op0=mybir.AluOpType.mult)

            # ---------------- Phase B: attention + Wo + RS -------------------
            bounce = []
            rs_out = []
            for c in range(NQS):
                bounce.append(dramp.tile([QS, D], f32, tag=f"bounce{c}", name=f"bounce{c}"))
                rs_out.append(dramp.tile([P, D], f32, tag=f"rsout{c}", name=f"rsout{c}"))

            with tc.tile_pool(name="mk", bufs=1) as mkp, \
                 tc.tile_pool(name="pstr", bufs=2) as pstrp, \
                 tc.tile_pool(name="ctx", bufs=1) as ctxp, \
                 tc.tile_pool(name="att_sc", bufs=2, space="PSUM") as scps, \
                 tc.tile_pool(name="att_pv", bufs=1, space="PSUM") as pvps, \
                 tc.tile_pool(name="att_d", bufs=1, space="PSUM") as dps, \
                 tc.tile_pool(name="att_wo", bufs=1, space="PSUM") as wops, \
                 tc.tile_pool(name="ostage", bufs=3) as ostage, \
                 tc.tile_pool(name="post", bufs=1) as postp:

                ctxT = ctxp.tile([P, DHG // P, S], bf16)

                for qs in range(NQS):
                    mT = mkp.tile([P, NTT, QS], bf16, tag="maskT")
                    for st in range(NTT):
                        nc.sync.dma_start(
                            mT[:, st],
                            dram_in["mask_bf"][qs * QS:(qs + 1) * QS,
                                               st * P:(st + 1) * P],
                            transpose=True)
                    for hp in range(2):
                        pstr2 = [pstrp.tile([P, NTT, QS], bf16, tag=f"pstr{i}", name=f"pstr{i}")
                                 for i in range(2)]
                        for st in range(NTT):
                            scs = [scps.tile([P, QS], f32, tag=f"scps{i}", name=f"scps{i}")
                                   for i in range(2)]
                            for hin in range(2):
                                nc.tensor.matmul(
                                    scs[hin][:],
                                    kT[hin * 64:(hin + 1) * 64, hp,
                                       st * P:(st + 1) * P],
                                    qT[hin * 64:(hin + 1) * 64, hp,
                                       qs * QS:(qs + 1) * QS],
                                    start=True, stop=True,
                                    tile_position=(hin * 64, 0))
                            for hin in range(2):
                                nc.scalar.activation(
                                    pstr2[hin][:, st], scs[hin][:],
                                    mybir.ActivationFunctionType.Exp,
                                    scale=1.0 / math.sqrt(DK))
                        for hin in range(2):
                            pstr = pstr2[hin]
                            h = hp * 2 + hin
                            # mask the whole strip in one op
                            nc.vector.tensor_mul(
                                pstr[:].rearrange("p t q -> p (t q)"),
                                pstr[:].rearrange("p t q -> p (t q)"),
                                mT[:].rearrange("p t q -> p (t q)"))
                            # PV with ones column -> [65, QS]
                            pv = pvps.tile([DK + 1, QS], f32, tag="pvps")
                            for st in range(NTT):
                                nc.tensor.matmul(
                                    pv[:],
                                    vhat[:, st, h, :],
                                    pstr[:, st],
                                    start=(st == 0), stop=(st == NTT - 1))
                            ce = ostage.tile([DK + 1, QS], f32, tag="ce")
                            nc.scalar.copy(ce[:], pv[:])
                            # normalize + re-transpose into ctxT
                            for blk in range(QS // P):
                                pt = dps.tile([P, DK + 1], f32, tag="dpt")
                                nc.tensor.transpose(
                                    pt[:], ce[:, blk * P:(blk + 1) * P],
                                    ident[0:DK + 1, 0:DK + 1])
                                rec = ostage.tile([P, 1], f32, tag="rec")
                                nc.vector.reciprocal(rec[:], pt[:, DK:DK + 1])
                                ctok = ostage.tile([P, DK], bf16, tag="ctok")
                                nc.scalar.activation(
                                    ctok[:], pt[:, 0:DK],
                                    mybir.ActivationFunctionType.Copy, scale=rec[:])
                                pb = dps.tile([DK, P], bf16, tag="dpb")
                                nc.tensor.transpose(pb[:], ctok[:], identb[:])
                                nc.scalar.copy(
                                    ctxT[hin * 64:hin * 64 + DK, hp,
                                         qs * QS + blk * P: qs * QS + (blk + 1) * P],
                                    pb[:])
                    # Wo for this q-slice
                    for t4 in range(QS // P):
                        tok0 = qs * QS + t4 * P
                        for nn in range(2):
                            wp = wops.tile([P, 512], f32, tag="wops")
                            for kt in range(DHG // P):
                                nc.tensor.matmul(
                                    wp[:],
                                    ctxT[:, kt, tok0:tok0 + P],
                                    wo_sb[:, kt, nn * 512:(nn + 1) * 512],
                                    start=(kt == 0), stop=(kt == DHG // P - 1))
                            ost = ostage.tile([P, 512], f32, tag="ost")
                            nc.scalar.copy(ost[:], wp[:])
                            nc.sync.dma_start(
                                bounce[qs][t4 * P:(t4 + 1) * P,
                                           nn * 512:(nn + 1) * 512], ost[:])
                    nc.gpsimd.collective_compute(
                        "ReduceScatter", mybir.AluOpType.add,
                        replica_groups=[[0, 1, 2, 3], [4, 5, 6, 7]],
                        ins=[bounce[qs].opt()], outs=[rs_out[qs].opt()])
                    # post-RS: residual vn for own rows + optional affine
                    vs = postp.tile([P, D], f32, tag="vs")
                    nc.sync.dma_start(vs[:], dram_in["v_own"][qs])
                    st2 = postp.tile([P, 2, 6], f32, tag="bn2")
                    vr = vs[:].rearrange("p (n f) -> p n f", f=512)
                    nc.vector.bn_stats(out=st2[:, 0], in_=vr[:, 0])
                    nc.vector.bn_stats(out=st2[:, 1], in_=vr[:, 1])
                    mv2 = postp.tile([P, 2], f32, tag="mv2")
                    nc.vector.bn_aggr(out=mv2[:], in_=st2[:])
                    sd2 = postp.tile([P, 1], f32, tag="sd2")
                    nc.scalar.activation(sd2[:], mv2[:, 1:2],
                                         mybir.ActivationFunctionType.Sqrt,
                                         scale=VAR_C)
                    nc.vector.tensor_scalar(out=sd2[:], in0=sd2[:], scalar1=EPS,
                                            scalar2=None, op0=mybir.AluOpType.add)
                    ri2 = postp.tile([P, 1], f32, tag="ri2")
                    nc.vector.reciprocal(ri2[:], sd2[:])
                    nm2 = postp.tile([P, 1], f32, tag="nm2")
                    nc.vector.tensor_scalar(out=nm2[:], in0=mv2[:, 0:1],
                                            scalar1=-1.0, scalar2=None,
                                            op0=mybir.AluOpType.mult)
                    vn = postp.tile([P, D], f32, tag="vn")
                    nc.vector.tensor_scalar(out=vn[:], in0=vs[:],
                                            scalar1=nm2[:], scalar2=ri2[:],
                                            op0=mybir.AluOpType.add,
                                            op1=mybir.AluOpType.mult)
                    if has_a2:
                        a2t = postp.tile([P, D], f32, tag="a2t")
                        nc.sync.dma_start(a2t[:], dram_in["a2f"].partition_broadcast(P))
                        nc.vector.tensor_mul(vn[:], vn[:], a2t[:])
                    ro = postp.tile([P, D], f32, tag="ro")
                    nc.sync.dma_start(ro[:], rs_out[qs][:])
                    nc.vector.tensor_add(ro[:], ro[:], vn[:])
                    if has_bias_out:
                        bot = postp.tile([P, D], f32, tag="bot")
                        nc.sync.dma_start(bot[:], dram_in["bias_out"].partition_broadcast(P))
                        nc.vector.tensor_add(ro[:], ro[:], bot[:])
                    nc.sync.dma_start(out_shard[qs], ro[:])

    nc.compile()
    return nc


def _prep_inputs(k, q, v, mask, Wq, bq, Wk, bk, Wv, bv, Wo, bo, a2, b2):
    """Host-side fold + shard. Returns list of per-core input dicts."""
    a2 = np.asarray(a2, np.float32); b2 = np.asarray(b2, np.float32)
    mask_bf = np.asarray(mask != 0).astype(BF)
    in_maps = []
    w_eff = {}
    for nm, W, bias in [("q", Wq, bq), ("k", Wk, bk), ("v", Wv, bv)]:
        We = (a2[:, None] * np.asarray(W, np.float32))
        be = b2 @ np.asarray(W, np.float32) + np.asarray(bias, np.float32)
        w_eff[nm] = (We, be)
    bias_out = (np.asarray(bo, np.float32)).reshape(1, D)
    for g in range(B):
        for r in range(HG):
            hsl = slice(r * DHG, (r + 1) * DHG)
            d = {
                "xk": np.ascontiguousarray(k[g]),
                "xq": np.ascontiguousarray(q[g]),
                "xv": np.ascontiguousarray(v[g]),
                "v_own": np.ascontiguousarray(
                    np.stack([v[g][c * QS + r * P: c * QS + (r + 1) * P]
                              for c in range(NQS)])),
                "mask_bf": np.ascontiguousarray(mask_bf[g]),
                "wo": np.ascontiguousarray(
                    np.asarray(Wo, np.float32)[hsl]).astype(BF),
                "a2f": a2.reshape(1, D),
                "bias_out": bias_out,
            }
            for nm in ["q", "k", "v"]:
                We, be = w_eff[nm]
                d[f"w{nm}"] = np.ascontiguousarray(We[:, hsl]).astype(BF)
                d[f"c{nm}"] = np.ascontiguousarray(
                    np.stack([We[:, hsl].sum(0), be[hsl]])).astype(BF)
            in_maps.append(d)
    return in_maps


def kernel(k, q, v, mask, Wq, bq, Wk, bk, Wv, bv, Wo, bo, a2, b2):
    k = np.asarray(k, np.float32); q = np.asarray(q, np.float32)
    v = np.asarray(v, np.float32); mask = np.asarray(mask)
    has_a2 = not np.allclose(np.asarray(a2, np.float32), 1.0)
    bias_out = np.asarray(bo, np.float32)
    has_bias_out = bool(np.any(bias_out != 0))
    key = (has_a2, has_bias_out)
    if key not in _CACHE:
        _CACHE[key] = _build(has_a2, has_bias_out)
    nc = _CACHE[key]
    in_maps = _prep_inputs(k, q, v, mask, Wq, bq, Wk, bk, Wv, bv, Wo, bo, a2, b2)
    res = run_bass_kernel_spmd(nc, in_maps, core_ids=list(range(NCORES)))
    out = np.empty((B, S, D), np.float32)
    ci = 0
    for g in range(B):
        for r in range(HG):
            sh = res.results[ci]["out_shard"]   # [NQS, P, D]
            for c in range(NQS):
                out[g, c * QS + r * P: c * QS + (r + 1) * P] = sh[c]
            ci += 1
    return out


if __name__ == "__main__":
    rng = np.random.default_rng(0)
    pass
